# revision 10
# baseline (speedup 1.0000x reference)
"""GCAT (GraphAttention + GraphConvolution) Bass kernel for 8 TRN2 NeuronCores.

Math (reference):
    h  = x @ W                                   [N, 200]
    e  = leaky_relu(h@a1 + (h@a2).T, 0.2)        [N, N]
    att = softmax(where(support > 0, e, -9e15), axis=1)
    hp = elu(att @ h)                            [N, 200]
    out = support @ (hp @ W0)                    [N, 20]

Distribution: row-shard the N dimension across 8 cores (R = N/8 rows each).
Each core holds S^T[:, rows] (the transposed row-block of support, prepared
host-side) so the contraction index j sits on SBUF partitions for both the
attention matmul and the GCN matmul — support is read from HBM exactly once
and kept resident in SBUF as bf16.

Key identities used on-device:
  exp(leaky_relu(z)) = max(exp(z), exp(0.2 z)) with z = f1_i + f2_j, and
  exp(0.2 z) = exp(0.2 f1_i) * exp(0.2 f2_j) (rank-1), so the NxN block needs
  only: one ACT pass (exp with per-partition bias), one fused DVE
  scalar_tensor_tensor (mult+max), one fused DVE mask pass ((S>0)*m).
  Softmax row sums come free as a ones-column appended to h in the attention
  matmul; normalization commutes with the (linear) matmul so it is applied
  after, per-partition.

Collectives: AllGather of [h|1] (bf16) + (f2, exp(0.2 f2)) (f32) after the
local h matmul, and AllGather of g = elu(U)@W0 before the GCN matmul.
"""

import numpy as np

import concourse.bass as bass
import concourse.tile as tile
from concourse import bacc, mybir
from concourse.bass_utils import run_bass_kernel_spmd
from concourse.masks import make_identity

N_CORES = 8
N_NODES = 8192
F_IN = 512
F_OUT = 200
C_OUT = 20
ALPHA = 0.2

F32 = mybir.dt.float32
BF16 = mybir.dt.bfloat16


def build_program(n_nodes=N_NODES, f_in=F_IN, f_out=F_OUT, c_out=C_OUT,
                  n_cores=N_CORES):
    R = n_nodes // n_cores          # rows (i) per core
    JT = n_nodes // 128             # j tiles (partition dim of S^T block)
    IS = R // 128                   # i subtiles per core
    KT = f_in // 128                # k tiles for the h matmul
    FA = f_out + 2                  # [h | f1 | f2] columns of the W_aug matmul
    HA = f_out + 1                  # [h | ones] gathered columns
    FL = f_out - 128                # low chunk of the feature dim (72)
    NH = (R + 511) // 512           # 512-wide output chunks for the GCN psum
    assert R % 128 == 0 and n_nodes % 128 == 0 and f_in % 128 == 0
    assert 128 < f_out <= 256 and HA <= 512

    nc = bacc.Bacc("TRN2", debug=False, num_devices=n_cores)
    xT = nc.dram_tensor("xT", [f_in, R], F32, kind="ExternalInput").ap()
    ST = nc.dram_tensor("ST", [n_nodes, R], F32, kind="ExternalInput").ap()
    Waug = nc.dram_tensor("Waug", [f_in, FA], F32, kind="ExternalInput").ap()
    W0 = nc.dram_tensor("W0", [f_out, c_out], F32, kind="ExternalInput").ap()
    outT = nc.dram_tensor("outT", [c_out, R], F32, kind="ExternalOutput").ap()

    rg = [list(range(n_cores))]

    with tile.TileContext(nc) as tc:
        with (
            tc.tile_pool(name="dram", bufs=1, space="DRAM") as dram,
            tc.tile_pool(name="res", bufs=1) as res,         # resident sbuf
            tc.tile_pool(name="stage", bufs=3) as stage,     # big f32 staging
            tc.tile_pool(name="work", bufs=2) as work,       # NxN elementwise
            tc.tile_pool(name="small", bufs=2) as small,     # per-i temps
            tc.tile_pool(name="ps", bufs=8, space="PSUM") as ps,
        ):
            # ---- DRAM bounce buffers for the collectives ----
            ag_h_in = dram.tile([R, HA], BF16)
            ag_h_out = dram.tile([n_nodes, HA], BF16, addr_space="Shared")
            ag_s_in = dram.tile([R, 2], F32)
            ag_s_out = dram.tile([n_nodes, 2], F32, addr_space="Shared")
            ag_g_in = dram.tile([R, c_out], F32)
            ag_g_out = dram.tile([n_nodes, c_out], F32, addr_space="Shared")
            f1_dram = dram.tile([R, 1], F32)

            # ---- resident SBUF ----
            stb = res.tile([128, JT * R], BF16)        # S^T block, bf16
            haug = res.tile([128, JT, HA], BF16)       # gathered [h | 1]
            svt = res.tile([128, JT, 2], F32)          # gathered (f2, e^{.2 f2})
            f1b = res.tile([128, R], F32)              # f1 broadcast (free dim)
            upb = res.tile([128, R], BF16)             # exp(0.2 f1) broadcast
            hp_sb = res.tile([128, IS, f_out], BF16)   # h' rows
            hpT_hi = res.tile([128, R], BF16)          # h'^T rows 0..127
            hpT_lo = res.tile([FL, R], BF16)           # h'^T rows 128..f_out
            gbf = res.tile([128, JT, c_out], BF16)     # gathered g, bf16
            ident = res.tile([128, 128], BF16)
            wk = res.tile([128, KT, FA], F32)          # W_aug, k-partitioned
            w0_hi = res.tile([128, c_out], BF16)
            w0_lo = res.tile([FL, c_out], BF16)

            make_identity(nc, ident[:])

            # ================= phase 0: h | f1 | f2 = x @ W_aug =============
            nc.sync.dma_start(wk[:], Waug.rearrange("(k p) c -> p k c", p=128))
            ph = [ps.tile([128, FA], F32, tag="ps", name=f"ph{i}")
                  for i in range(IS)]
            for k in range(KT):
                xk = stage.tile([128, R], F32, tag="stage", name=f"xk{k}")
                nc.sync.dma_start(xk[:], xT[k * 128:(k + 1) * 128, :])
                for i in range(IS):
                    nc.tensor.matmul(ph[i][:], lhsT=xk[:, i * 128:(i + 1) * 128],
                                     rhs=wk[:, k, :],
                                     start=(k == 0), stop=(k == KT - 1))
            for i in range(IS):
                h_loc = small.tile([128, HA], BF16, tag="h_loc")
                nc.scalar.copy(h_loc[:, 0:f_out], ph[i][:, 0:f_out])
                nc.gpsimd.memset(h_loc[:, f_out:HA], 1.0)
                nc.sync.dma_start(ag_h_in[i * 128:(i + 1) * 128, :], h_loc[:])
                sv_loc = small.tile([128, 2], F32, tag="sv_loc")
                nc.scalar.copy(sv_loc[:, 0:1], ph[i][:, f_out + 1:f_out + 2])
                nc.scalar.activation(sv_loc[:, 1:2], ph[i][:, f_out + 1:f_out + 2],
                                     mybir.ActivationFunctionType.Exp, scale=ALPHA)
                nc.sync.dma_start(ag_s_in[i * 128:(i + 1) * 128, :], sv_loc[:])
                f1_loc = small.tile([128, 1], F32, tag="f1_loc")
                nc.scalar.copy(f1_loc[:], ph[i][:, f_out:f_out + 1])
                nc.sync.dma_start(f1_dram[i * 128:(i + 1) * 128, :], f1_loc[:])

            # f1 broadcast along the free dim on every partition
            f1_bc_src = bass.AP(tensor=f1_dram.tensor, offset=f1_dram.offset,
                                ap=[[0, 128], [1, R]])
            nc.sync.dma_start(f1b[:], f1_bc_src)
            nc.scalar.activation(upb[:], f1b[:],
                                 mybir.ActivationFunctionType.Exp, scale=ALPHA)

            # ============= S^T stream-in + cast to resident bf16 ============
            for t in range(JT):
                stf = stage.tile([128, R], F32, tag="stage", name=f"stf{t}")
                nc.sync.dma_start(stf[:], ST[t * 128:(t + 1) * 128, :])
                nc.vector.tensor_copy(stb[:, t * R:(t + 1) * R], stf[:])

            # ======================= collectives 1+2 ========================
            nc.gpsimd.collective_compute(
                "AllGather", mybir.AluOpType.bypass, replica_groups=rg,
                ins=[ag_h_in.opt()], outs=[ag_h_out.opt()])
            nc.gpsimd.collective_compute(
                "AllGather", mybir.AluOpType.bypass, replica_groups=rg,
                ins=[ag_s_in.opt()], outs=[ag_s_out.opt()])
            nc.sync.dma_start(haug[:],
                              ag_h_out.rearrange("(t p) c -> p t c", p=128))
            nc.sync.dma_start(svt[:],
                              ag_s_out.rearrange("(t p) c -> p t c", p=128))

            # ==================== attention over j tiles ====================
            U = [ps.tile([128, HA], F32, tag="ps", name=f"U{i}")
                 for i in range(IS)]
            for t in range(JT):
                e1 = work.tile([128, R], BF16, tag="e1")
                nc.scalar.activation(e1[:], f1b[:],
                                     mybir.ActivationFunctionType.Exp,
                                     bias=svt[:, t, 0:1], scale=1.0)
                m = work.tile([128, R], BF16, tag="m")
                nc.vector.scalar_tensor_tensor(
                    m[:], in0=upb[:], scalar=svt[:, t, 1:2], in1=e1[:],
                    op0=mybir.AluOpType.mult, op1=mybir.AluOpType.max)
                p = work.tile([128, R], BF16, tag="p")
                nc.vector.scalar_tensor_tensor(
                    p[:], in0=stb[:, t * R:(t + 1) * R], scalar=0.0, in1=m[:],
                    op0=mybir.AluOpType.is_gt, op1=mybir.AluOpType.mult)
                for i in range(IS):
                    nc.tensor.matmul(U[i][:], lhsT=p[:, i * 128:(i + 1) * 128],
                                     rhs=haug[:, t, :],
                                     start=(t == 0), stop=(t == JT - 1))

            # ============ normalize + elu -> h', consume U tiles ============
            for i in range(IS):
                rec = small.tile([128, 1], F32, tag="rec")
                nc.vector.reciprocal(rec[:], U[i][:, f_out:HA])
                hp32 = small.tile([128, f_out], F32, tag="hp32", bufs=1)
                nc.vector.tensor_scalar_mul(hp32[:], U[i][:, 0:f_out], rec[:])
                neg = small.tile([128, f_out], F32, tag="neg", bufs=1)
                nc.vector.tensor_scalar(neg[:], hp32[:], 0.0, None,
                                        op0=mybir.AluOpType.min)
                expn = small.tile([128, f_out], F32, tag="expn", bufs=1)
                nc.scalar.activation(expn[:], neg[:],
                                     mybir.ActivationFunctionType.Exp)
                poz = small.tile([128, f_out], F32, tag="poz", bufs=1)
                nc.vector.tensor_scalar(poz[:], hp32[:], 0.0, None,
                                        op0=mybir.AluOpType.max)
                nc.vector.scalar_tensor_tensor(
                    hp_sb[:, i, :], in0=expn[:], scalar=-1.0, in1=poz[:],
                    op0=mybir.AluOpType.add, op1=mybir.AluOpType.add)

            # ================= g = h' @ W0  (local rows) ====================
            w0s = small.tile([128, 2 * c_out], F32, tag="w0s", bufs=1)
            nc.sync.dma_start(w0s[:, 0:c_out], W0[0:128, :])
            nc.sync.dma_start(w0s[0:FL, c_out:2 * c_out], W0[128:f_out, :])
            nc.scalar.copy(w0_hi[:], w0s[:, 0:c_out])
            nc.scalar.copy(w0_lo[:], w0s[0:FL, c_out:2 * c_out])
            for i in range(IS):
                pa = ps.tile([128, 128], BF16, tag="ps", name=f"pa{i}")
                nc.tensor.transpose(pa[:], hp_sb[:, i, 0:128], ident[:])
                nc.scalar.copy(hpT_hi[:, i * 128:(i + 1) * 128], pa[:])
                pb = ps.tile([FL, 128], BF16, tag="ps", name=f"pb{i}")
                nc.tensor.transpose(pb[:], hp_sb[:, i, 128:f_out], ident[:])
                nc.scalar.copy(hpT_lo[:, i * 128:(i + 1) * 128], pb[:])
                gp = ps.tile([128, c_out], F32, tag="ps", name=f"gp{i}")
                nc.tensor.matmul(gp[:], lhsT=hpT_hi[:, i * 128:(i + 1) * 128],
                                 rhs=w0_hi[:], start=True, stop=False)
                nc.tensor.matmul(gp[:], lhsT=hpT_lo[:, i * 128:(i + 1) * 128],
                                 rhs=w0_lo[:], start=False, stop=True)
                gl = small.tile([128, c_out], F32, tag="gl")
                nc.scalar.copy(gl[:], gp[:])
                nc.sync.dma_start(ag_g_in[i * 128:(i + 1) * 128, :], gl[:])

            # ======================= collective 3 ===========================
            nc.gpsimd.collective_compute(
                "AllGather", mybir.AluOpType.bypass, replica_groups=rg,
                ins=[ag_g_in.opt()], outs=[ag_g_out.opt()])
            JTH = JT // 2
            g_src = ag_g_out.rearrange("(t p) c -> p t c", p=128)
            for hh in range(2):
                gfl = stage.tile([128, JTH, c_out], F32, tag="stage",
                                 name=f"gfl{hh}")
                nc.sync.dma_start(gfl[:], g_src[:, hh * JTH:(hh + 1) * JTH, :])
                nc.vector.tensor_copy(gbf[:, hh * JTH:(hh + 1) * JTH, :],
                                      gfl[:])

            # ================= GCN: out^T = sum_j g_j S^T_j =================
            og = [ps.tile([c_out, min(512, R - h * 512)], F32, tag="ps",
                          name=f"og{h}") for h in range(NH)]
            for t in range(JT):
                for h in range(NH):
                    w = min(512, R - h * 512)
                    nc.tensor.matmul(
                        og[h][:], lhsT=gbf[:, t, :],
                        rhs=stb[:, t * R + h * 512: t * R + h * 512 + w],
                        start=(t == 0), stop=(t == JT - 1))
            for h in range(NH):
                w = min(512, R - h * 512)
                out_sb = small.tile([c_out, 512], F32, tag="out_sb", bufs=2)
                nc.scalar.copy(out_sb[:, 0:w], og[h][:])
                nc.sync.dma_start(outT[:, h * 512:h * 512 + w], out_sb[:, 0:w])

    nc.compile()
    return nc


_PROGRAM_CACHE = {}
TRACE = False        # set by the dev harness to capture an NTFF profile
LAST_RESULT = None   # BassKernelResults of the most recent run


def _get_program(key):
    if key not in _PROGRAM_CACHE:
        _PROGRAM_CACHE[key] = build_program(*key)
    return _PROGRAM_CACHE[key]


def kernel(x, support, W, a1, a2, W0):
    n, f_in = x.shape
    f_out = W.shape[1]
    c_out = W0.shape[1]
    R = n // N_CORES

    nc = _get_program((n, f_in, f_out, c_out, N_CORES))

    # Host-side layout prep (sharding): transpose so the contraction index of
    # every device matmul lands on the SBUF partition axis.
    x = np.ascontiguousarray(x, dtype=np.float32)
    support = np.ascontiguousarray(support, dtype=np.float32)
    xT = x.T                                  # [f_in, n]
    STt = support.T                           # [n, n]; core m takes cols m*R..
    Waug = np.concatenate([W, W @ a1, W @ a2], axis=1).astype(np.float32)
    Waug = np.ascontiguousarray(Waug)
    W0c = np.ascontiguousarray(W0, dtype=np.float32)

    in_maps = []
    for m in range(N_CORES):
        in_maps.append({
            "xT": np.ascontiguousarray(xT[:, m * R:(m + 1) * R]),
            "ST": np.ascontiguousarray(STt[:, m * R:(m + 1) * R]),
            "Waug": Waug,
            "W0": W0c,
        })

    res = run_bass_kernel_spmd(nc, in_maps, list(range(N_CORES)), trace=TRACE)
    global LAST_RESULT
    LAST_RESULT = res

    out = np.empty((n, c_out), dtype=np.float32)
    for m in range(N_CORES):
        out[m * R:(m + 1) * R, :] = res.results[m]["outT"].T
    return out


# revision 14
# speedup vs baseline: 1.0695x; 1.0695x over previous
"""GCAT (GraphAttention + GraphConvolution) Bass kernel for 8 TRN2 NeuronCores.

Math (reference):
    h  = x @ W                                   [N, 200]
    e  = leaky_relu(h@a1 + (h@a2).T, 0.2)        [N, N]
    att = softmax(where(support > 0, e, -9e15), axis=1)
    hp = elu(att @ h)                            [N, 200]
    out = support @ (hp @ W0)                    [N, 20]

Distribution: row-shard the N dimension across 8 cores (R = N/8 rows each).
Each core holds S^T[:, rows] (the transposed row-block of support, prepared
host-side) so the contraction index j sits on SBUF partitions for both the
attention matmul and the GCN matmul — support is read from HBM exactly once
and kept resident in SBUF as bf16.

Key identities used on-device:
  exp(leaky_relu(z)) = max(exp(z), exp(0.2 z)) with z = f1_i + f2_j, and
  exp(0.2 z) = exp(0.2 f1_i) * exp(0.2 f2_j) (rank-1), so the NxN block needs
  only: one ACT pass (exp with per-partition bias), one fused DVE
  scalar_tensor_tensor (mult+max), one fused DVE mask pass ((S>0)*m).
  Softmax row sums come free as a ones-column appended to h in the attention
  matmul; normalization commutes with the (linear) matmul so it is applied
  after, per-partition.

Collectives: AllGather of [h|1] (bf16) + (f2, exp(0.2 f2)) (f32) after the
local h matmul, and AllGather of g = elu(U)@W0 before the GCN matmul.
"""

import numpy as np

import concourse.bass as bass
import concourse.tile as tile
from concourse import bacc, mybir
from concourse.bass_utils import run_bass_kernel_spmd
from concourse.masks import make_identity

N_CORES = 8
N_NODES = 8192
F_IN = 512
F_OUT = 200
C_OUT = 20
ALPHA = 0.2

F32 = mybir.dt.float32
BF16 = mybir.dt.bfloat16


def build_program(n_nodes=N_NODES, f_in=F_IN, f_out=F_OUT, c_out=C_OUT,
                  n_cores=N_CORES):
    R = n_nodes // n_cores          # rows (i) per core
    JT = n_nodes // 128             # j tiles (partition dim of S^T block)
    IS = R // 128                   # i subtiles per core
    KT = f_in // 128                # k tiles for the h matmul
    FA = f_out + 2                  # [h | f1 | f2] columns of the W_aug matmul
    HA = f_out + 1                  # [h | ones] matmul-rhs columns
    HG = f_out + 8                  # gathered row: h|1|pad|f2bits|f2sbits|pad
    FL = f_out - 128                # low chunk of the feature dim (72)
    NH = (R + 511) // 512           # 512-wide output chunks for the GCN psum
    assert R % 128 == 0 and n_nodes % 128 == 0 and f_in % 128 == 0
    assert 128 < f_out <= 256 and HA <= 512

    nc = bacc.Bacc("TRN2", debug=False, num_devices=n_cores)
    xT = nc.dram_tensor("xT", [f_in, R], F32, kind="ExternalInput").ap()
    ST = nc.dram_tensor("ST", [n_nodes, R], F32, kind="ExternalInput").ap()
    Waug = nc.dram_tensor("Waug", [f_in, FA], F32, kind="ExternalInput").ap()
    W0 = nc.dram_tensor("W0", [f_out, c_out], F32, kind="ExternalInput").ap()
    outT = nc.dram_tensor("outT", [c_out, R], F32, kind="ExternalOutput").ap()

    rg = [list(range(n_cores))]

    with tile.TileContext(nc) as tc:
        with (
            tc.tile_pool(name="dram", bufs=1, space="DRAM") as dram,
            tc.tile_pool(name="res", bufs=1) as res,         # resident sbuf
            tc.tile_pool(name="stage", bufs=3) as stage,     # big f32 staging
            tc.tile_pool(name="work", bufs=2) as work,       # NxN elementwise
            tc.tile_pool(name="small", bufs=2) as small,     # per-i temps
            tc.tile_pool(name="ps", bufs=8, space="PSUM") as ps,
        ):
            # ---- DRAM bounce buffers for the collectives ----
            ag_h_in = dram.tile([R, HG], BF16)
            ag_h_out = dram.tile([n_nodes, HG], BF16, addr_space="Shared")
            ag_g_in = dram.tile([R, c_out], BF16)
            ag_g_out = dram.tile([n_nodes, c_out], BF16, addr_space="Shared")
            f1_dram = dram.tile([R, 1], F32)

            # ---- resident SBUF ----
            stb = res.tile([128, JT * R], BF16)        # S^T block, bf16
            haug = res.tile([128, JT, HG], BF16)       # gathered [h|1|f2|f2s]
            f1b = res.tile([128, R], F32)              # f1 broadcast (free dim)
            hp_sb = res.tile([128, IS, f_out], BF16)   # h' rows
            hpT_hi = res.tile([128, R], BF16)          # h'^T rows 0..127
            hpT_lo = res.tile([FL, R], BF16)           # h'^T rows 128..f_out
            gbf = res.tile([128, JT, c_out], BF16)     # gathered g, bf16
            ident = res.tile([128, 128], BF16)
            wk = res.tile([128, KT, FA], F32)          # W_aug, k-partitioned
            w0_hi = res.tile([128, c_out], BF16)
            w0_lo = res.tile([FL, c_out], BF16)

            make_identity(nc, ident[:])

            # ================= phase 0: h | f1 | f2 = x @ W_aug =============
            nc.sync.dma_start(wk[:], Waug.rearrange("(k p) c -> p k c", p=128))
            ph = [ps.tile([128, FA], F32, tag="ps", name=f"ph{i}")
                  for i in range(IS)]
            for k in range(KT):
                xk = stage.tile([128, R], F32, tag="stage", name=f"xk{k}")
                nc.sync.dma_start(xk[:], xT[k * 128:(k + 1) * 128, :])
                for i in range(IS):
                    nc.tensor.matmul(ph[i][:], lhsT=xk[:, i * 128:(i + 1) * 128],
                                     rhs=wk[:, k, :],
                                     start=(k == 0), stop=(k == KT - 1))
            for i in range(IS):
                h_loc = small.tile([128, HG], BF16, tag="h_loc")
                nc.scalar.copy(h_loc[:, 0:f_out], ph[i][:, 0:f_out])
                nc.gpsimd.memset(h_loc[:, f_out:f_out + 2], 1.0)
                sv_loc = small.tile([128, 2], F32, tag="sv_loc")
                nc.scalar.copy(sv_loc[:, 0:1], ph[i][:, f_out + 1:f_out + 2])
                nc.scalar.mul(sv_loc[:, 1:2], ph[i][:, f_out + 1:f_out + 2], ALPHA)
                nc.vector.tensor_copy(h_loc[:, f_out + 2:f_out + 6].bitcast(F32),
                                      sv_loc[:])
                nc.gpsimd.memset(h_loc[:, f_out + 6:HG], 0.0)
                nc.sync.dma_start(ag_h_in[i * 128:(i + 1) * 128, :], h_loc[:])
                f1_loc = small.tile([128, 1], F32, tag="f1_loc")
                nc.scalar.copy(f1_loc[:], ph[i][:, f_out:f_out + 1])
                nc.sync.dma_start(f1_dram[i * 128:(i + 1) * 128, :], f1_loc[:])

            # f1 broadcast along the free dim on every partition
            f1_bc_src = bass.AP(tensor=f1_dram.tensor, offset=f1_dram.offset,
                                ap=[[0, 128], [1, R]])
            nc.sync.dma_start(f1b[:], f1_bc_src)

            # ============= S^T stream-in + cast to resident bf16 ============
            dma_engs = [nc.sync, nc.gpsimd, nc.scalar]
            for t in range(JT):
                stf = stage.tile([128, R], F32, tag="stage", name=f"stf{t}")
                dma_engs[t % 3].dma_start(stf[:], ST[t * 128:(t + 1) * 128, :])
                nc.vector.tensor_copy(stb[:, t * R:(t + 1) * R], stf[:])

            # ======================= collectives 1+2 ========================
            nc.gpsimd.collective_compute(
                "AllGather", mybir.AluOpType.bypass, replica_groups=rg,
                ins=[ag_h_in.opt()], outs=[ag_h_out.opt()])
            nc.sync.dma_start(haug[:],
                              ag_h_out.rearrange("(t p) c -> p t c", p=128))

            # ==================== attention over j tiles ====================
            U = [ps.tile([128, HA], F32, tag="ps", name=f"U{i}")
                 for i in range(IS)]
            fview = haug[:, :, f_out + 2:f_out + 6].bitcast(F32)
            for t in range(JT):
                e1 = work.tile([128, R], BF16, tag="e1")
                nc.scalar.activation(e1[:], f1b[:],
                                     mybir.ActivationFunctionType.Exp,
                                     bias=fview[:, t, 0:1], scale=1.0)
                e2 = work.tile([128, R], BF16, tag="e2")
                nc.scalar.activation(e2[:], f1b[:],
                                     mybir.ActivationFunctionType.Exp,
                                     bias=fview[:, t, 1:2], scale=ALPHA)
                m = work.tile([128, R], BF16, tag="m")
                nc.vector.tensor_tensor(m[:], e1[:], e2[:],
                                        op=mybir.AluOpType.max)
                p = work.tile([128, R], BF16, tag="p")
                nc.vector.scalar_tensor_tensor(
                    p[:], in0=stb[:, t * R:(t + 1) * R], scalar=0.0, in1=m[:],
                    op0=mybir.AluOpType.is_gt, op1=mybir.AluOpType.mult)
                for i in range(IS):
                    nc.tensor.matmul(U[i][:], lhsT=p[:, i * 128:(i + 1) * 128],
                                     rhs=haug[:, t, 0:HA],
                                     start=(t == 0), stop=(t == JT - 1))

            # ============ normalize + elu -> h', consume U tiles ============
            for i in range(IS):
                rec = small.tile([128, 1], F32, tag="rec")
                nc.vector.reciprocal(rec[:], U[i][:, f_out:HA])
                hp32 = small.tile([128, f_out], F32, tag="hp32", bufs=1)
                nc.vector.tensor_scalar_mul(hp32[:], U[i][:, 0:f_out], rec[:])
                neg = small.tile([128, f_out], F32, tag="neg", bufs=1)
                nc.vector.tensor_scalar(neg[:], hp32[:], 0.0, None,
                                        op0=mybir.AluOpType.min)
                expn = small.tile([128, f_out], F32, tag="expn", bufs=1)
                nc.scalar.activation(expn[:], neg[:],
                                     mybir.ActivationFunctionType.Exp)
                poz = small.tile([128, f_out], F32, tag="poz", bufs=1)
                nc.vector.tensor_scalar(poz[:], hp32[:], 0.0, None,
                                        op0=mybir.AluOpType.max)
                nc.vector.scalar_tensor_tensor(
                    hp_sb[:, i, :], in0=expn[:], scalar=-1.0, in1=poz[:],
                    op0=mybir.AluOpType.add, op1=mybir.AluOpType.add)

            # ================= g = h' @ W0  (local rows) ====================
            w0s = small.tile([128, 2 * c_out], F32, tag="w0s", bufs=1)
            nc.sync.dma_start(w0s[:, 0:c_out], W0[0:128, :])
            nc.sync.dma_start(w0s[0:FL, c_out:2 * c_out], W0[128:f_out, :])
            nc.scalar.copy(w0_hi[:], w0s[:, 0:c_out])
            nc.scalar.copy(w0_lo[:], w0s[0:FL, c_out:2 * c_out])
            for i in range(IS):
                pa = ps.tile([128, 128], BF16, tag="ps", name=f"pa{i}")
                nc.tensor.transpose(pa[:], hp_sb[:, i, 0:128], ident[:])
                nc.scalar.copy(hpT_hi[:, i * 128:(i + 1) * 128], pa[:])
                pb = ps.tile([FL, 128], BF16, tag="ps", name=f"pb{i}")
                nc.tensor.transpose(pb[:], hp_sb[:, i, 128:f_out], ident[:])
                nc.scalar.copy(hpT_lo[:, i * 128:(i + 1) * 128], pb[:])
                gp = ps.tile([128, c_out], F32, tag="ps", name=f"gp{i}")
                nc.tensor.matmul(gp[:], lhsT=hpT_hi[:, i * 128:(i + 1) * 128],
                                 rhs=w0_hi[:], start=True, stop=False)
                nc.tensor.matmul(gp[:], lhsT=hpT_lo[:, i * 128:(i + 1) * 128],
                                 rhs=w0_lo[:], start=False, stop=True)
                gl = small.tile([128, c_out], BF16, tag="gl")
                nc.scalar.copy(gl[:], gp[:])
                nc.sync.dma_start(ag_g_in[i * 128:(i + 1) * 128, :], gl[:])

            # ======================= collective 3 ===========================
            nc.gpsimd.collective_compute(
                "AllGather", mybir.AluOpType.bypass, replica_groups=rg,
                ins=[ag_g_in.opt()], outs=[ag_g_out.opt()])
            nc.sync.dma_start(gbf[:],
                              ag_g_out.rearrange("(t p) c -> p t c", p=128))

            # ================= GCN: out^T = sum_j g_j S^T_j =================
            og = [ps.tile([c_out, min(512, R - h * 512)], F32, tag="ps",
                          name=f"og{h}") for h in range(NH)]
            for t in range(JT):
                for h in range(NH):
                    w = min(512, R - h * 512)
                    nc.tensor.matmul(
                        og[h][:], lhsT=gbf[:, t, :],
                        rhs=stb[:, t * R + h * 512: t * R + h * 512 + w],
                        start=(t == 0), stop=(t == JT - 1))
            for h in range(NH):
                w = min(512, R - h * 512)
                out_sb = small.tile([c_out, 512], F32, tag="out_sb", bufs=2)
                nc.scalar.copy(out_sb[:, 0:w], og[h][:])
                nc.sync.dma_start(outT[:, h * 512:h * 512 + w], out_sb[:, 0:w])

    nc.compile()
    return nc


_PROGRAM_CACHE = {}
TRACE = False        # set by the dev harness to capture an NTFF profile
LAST_RESULT = None   # BassKernelResults of the most recent run


def _get_program(key):
    if key not in _PROGRAM_CACHE:
        _PROGRAM_CACHE[key] = build_program(*key)
    return _PROGRAM_CACHE[key]


def kernel(x, support, W, a1, a2, W0):
    n, f_in = x.shape
    f_out = W.shape[1]
    c_out = W0.shape[1]
    R = n // N_CORES

    nc = _get_program((n, f_in, f_out, c_out, N_CORES))

    # Host-side layout prep (sharding): transpose so the contraction index of
    # every device matmul lands on the SBUF partition axis.
    x = np.ascontiguousarray(x, dtype=np.float32)
    support = np.ascontiguousarray(support, dtype=np.float32)
    xT = x.T                                  # [f_in, n]
    STt = support.T                           # [n, n]; core m takes cols m*R..
    Waug = np.concatenate([W, W @ a1, W @ a2], axis=1).astype(np.float32)
    Waug = np.ascontiguousarray(Waug)
    W0c = np.ascontiguousarray(W0, dtype=np.float32)

    in_maps = []
    for m in range(N_CORES):
        in_maps.append({
            "xT": np.ascontiguousarray(xT[:, m * R:(m + 1) * R]),
            "ST": np.ascontiguousarray(STt[:, m * R:(m + 1) * R]),
            "Waug": Waug,
            "W0": W0c,
        })

    res = run_bass_kernel_spmd(nc, in_maps, list(range(N_CORES)), trace=TRACE)
    global LAST_RESULT
    LAST_RESULT = res

    out = np.empty((n, c_out), dtype=np.float32)
    for m in range(N_CORES):
        out[m * R:(m + 1) * R, :] = res.results[m]["outT"].T
    return out
